# revision 2
# baseline (speedup 1.0000x reference)
"""DFFN Trainium2 kernel v3: restructured for TimelineSim cost model.

Data-parallel over batch: 8 images, one per NeuronCore.

Pipeline per 16-row band (vs the v1 per-channel-matmul design):
  - input loads via gpsimd cast-DMA straight to bf16 (no fp32 copy in SBUF)
  - proj_in runs flipped (patch-pair chunks stationary) -> pixel comps on
    PSUM partitions
  - the patch FFT*filt*iFFT factorizes as G80 . diag(s_c) . F80 with
    CHANNEL-SHARED 80-dim real-spectrum transforms; only diag(s_c) is
    per-channel and it is a single elementwise multiply (B2). This replaces
    2048 per-channel matmuls with 48 shared-weight matmuls per band.
  - T' merges the inverse transform with the return-to-channel-partitions
    transpose: lhsT = scaled spectrum chunk, rhs = G80^T moving.
  - gelu reads PSUM x1 rows, gate multiplies by x2 rows and scatters
    DIRECTLY into the halo slab (no separate transpose eviction).
  - depthwise 3x3 folds into the before_dwconv matmul: psW accumulates 9
    matmuls of diag(wdw_k)@w_before against shifted slab windows.
  - residual rides the w_out PSUM group as an extra identity matmul on the
    bf16 input; a single Act eviction writes the final fp32 band.
"""

import sys

sys.path.insert(0, "/opt/trn_rl_repo")

import numpy as np
import ml_dtypes
from contextlib import ExitStack

import concourse.bass as bass
import concourse.mybir as mybir
import concourse.tile as tile
from concourse.bass_utils import run_bass_kernel_spmd
from concourse.masks import make_identity

F32 = mybir.dt.float32
BF16 = mybir.dt.bfloat16
F8 = mybir.dt.float8e4
BF = ml_dtypes.bfloat16
F8NP = ml_dtypes.float8_e4m3fn

B, C, H, W = 8, 128, 256, 256
HALF = C // 2
P = 8
BAND = 16
N_CORES = 8


# --------------------------------------------------------------------------
# host-side weight preprocessing
# --------------------------------------------------------------------------

def _f80_g80():
    """Shared real-spectrum transforms: F80 [80,64] pixel->spectrum slots,
    G80 [64,80] spectrum slots->pixel. Slot s=(u*5+v)*2+ri."""
    E = np.eye(P * P).reshape(P * P, P, P)
    FB = np.fft.rfft2(E)                       # [64, 8, 5]
    F80 = np.zeros((80, 64))
    G80 = np.zeros((64, 80))
    for u in range(8):
        for v in range(5):
            for ri in range(2):
                s = (u * 5 + v) * 2 + ri
                F80[s, :] = FB[:, u, v].real if ri == 0 else FB[:, u, v].imag
                d = np.zeros((8, 5), dtype=complex)
                d[u, v] = 1.0 if ri == 0 else 1.0j
                G80[:, s] = np.fft.irfft2(d, s=(P, P)).reshape(64)
    return F80, G80


def _prep_weights(fft_filt, w_in, w_before, w_dw, w_out):
    F80, G80 = _f80_g80()

    winT = np.ascontiguousarray(w_in.T)                   # [cin, cout]

    f80l = np.vstack([F80.T, F80.T])                      # [128, 80]

    s80 = np.zeros((80, C))                               # [slot, c]
    for u in range(8):
        for v in range(5):
            for ri in range(2):
                s80[(u * 5 + v) * 2 + ri, :] = fft_filt[:, u, v]
    s80t = np.tile(s80, (1, 4))                           # [80, 512]

    g80m = np.ascontiguousarray(G80.T)                    # [80, 64]

    # 9 taps: Wk = diag(wdw[:,k]) @ w_before ; lhsT = blockdiag(Wk^T, Wk^T).
    # fp8 DoubleRow pairs taps (0,1),(2,3),(4,5),(6,7) two-major + single 8.
    # Weights scaled x64 into fp8 normal range; woT2 carries the /64.
    wdw9 = w_dw.reshape(HALF, 9)
    wk1 = np.zeros((9, 128, 128))
    for k in range(9):
        WkT = (wdw9[:, k:k + 1] * w_before).T             # [cin=64, cout=64]
        wk1[k, 0:64, 0:64] = WkT
        wk1[k, 64:128, 64:128] = WkT
    wkp = np.zeros((128, 9 * 128))
    for j in range(3):
        wkp[:, j * 256:j * 256 + 128] = wk1[j] * 64.0
        wkp[:, j * 256 + 128:j * 256 + 256] = wk1[j + 3] * 64.0
    for j in range(3):
        wkp[:, 768 + j * 128:768 + (j + 1) * 128] = wk1[6 + j] * 64.0

    woT2 = np.vstack([w_out.T, w_out.T]) / 64.0           # [128, 128]

    return (winT.astype(BF), f80l.astype(BF), s80t.astype(BF),
            g80m.astype(BF), wkp.astype(F8NP), woT2.astype(BF))


# --------------------------------------------------------------------------
# the tile kernel (per core, one image)
# --------------------------------------------------------------------------

def build_kernel(nc, n_rows=H, legalize=True):
    x_d = nc.dram_tensor("x", [C, n_rows, W], F32, kind="ExternalInput").ap()
    winT_d = nc.dram_tensor("winT", [128, 128], BF16, kind="ExternalInput").ap()
    f80l_d = nc.dram_tensor("f80l", [128, 80], BF16, kind="ExternalInput").ap()
    s80t_d = nc.dram_tensor("s80t", [80, 512], BF16, kind="ExternalInput").ap()
    g80m_d = nc.dram_tensor("g80m", [80, 64], BF16, kind="ExternalInput").ap()
    wk9_d = nc.dram_tensor("wk9", [128, 9 * 128], F8, kind="ExternalInput").ap()
    woT2_d = nc.dram_tensor("woT2", [128, 128], BF16, kind="ExternalInput").ap()
    out_d = nc.dram_tensor("out", [C, n_rows, W], F32, kind="ExternalOutput").ap()

    n_bands = n_rows // BAND
    MULT = mybir.AluOpType.mult

    with tile.TileContext(nc) as tc, ExitStack() as ctx:
        singles = ctx.enter_context(tc.tile_pool(name="singles", bufs=1))
        xbf_p = ctx.enter_context(tc.tile_pool(name="xbf", bufs=4))
        xpat_p = ctx.enter_context(tc.tile_pool(name="xpat", bufs=3))
        abuf_p = ctx.enter_context(tc.tile_pool(name="abuf", bufs=3))
        yf_p = ctx.enter_context(tc.tile_pool(name="yf", bufs=2))
        gelu_p = ctx.enter_context(tc.tile_pool(name="gelu", bufs=4))
        gw_p = ctx.enter_context(tc.tile_pool(name="gw", bufs=3))
        slab_p = ctx.enter_context(tc.tile_pool(name="slab", bufs=4))
        outb_p = ctx.enter_context(tc.tile_pool(name="outb", bufs=2))

        psAT_p = ctx.enter_context(tc.tile_pool(name="psAT", bufs=2, space="PSUM"))
        psB_p = ctx.enter_context(tc.tile_pool(name="psB", bufs=2, space="PSUM"))
        psWO_p = ctx.enter_context(tc.tile_pool(name="psWO", bufs=2, space="PSUM"))

        # ---- weights ----
        winT = singles.tile([128, 128], BF16)
        nc.sync.dma_start(out=winT, in_=winT_d)
        f80l = singles.tile([128, 80], BF16)
        nc.sync.dma_start(out=f80l, in_=f80l_d)
        s80t = singles.tile([80, 512], BF16)
        nc.sync.dma_start(out=s80t, in_=s80t_d)
        g80m = singles.tile([80, 64], BF16)
        nc.sync.dma_start(out=g80m, in_=g80m_d)
        wk9 = singles.tile([128, 9 * 128], F8)
        nc.sync.dma_start(out=wk9, in_=wk9_d)
        woT2 = singles.tile([128, 128], BF16)
        nc.sync.dma_start(out=woT2, in_=woT2_d)
        ident = singles.tile([128, 128], BF16)
        make_identity(nc, ident)

        xbfs = []
        slabs = []

        def front(t):
            y0 = t * BAND
            # 0. cast-DMA: DRAM fp32 -> SBUF bf16, row-major
            xbf = xbf_p.tile([128, BAND * W], BF16)
            nc.gpsimd.dma_start(out=xbf, in_=x_d[:, y0:y0 + BAND, :])
            xbfs.append(xbf)

            # reorder to patch-pair-major: xpat col = pp*128 + pl*64 + i*8 + j
            xpat = xpat_p.tile([128, BAND * W], BF16)
            for h2 in range(2):
                src = bass.AP(tensor=xbf.tensor, offset=xbf.offset + h2 * 2048,
                              ap=[xbf.ap[0], [16, 16], [8, 2], [256, 8], [1, 8]])
                dst = bass.AP(tensor=xpat.tensor, offset=xpat.offset + h2 * 2048,
                              ap=[xpat.ap[0], [128, 16], [64, 2], [8, 8], [1, 8]])
                nc.gpsimd.tensor_copy(dst, src)

            # A / B1+B2 / T'+gelu+gate, software-pipelined on PE
            abuf = abuf_p.tile([128, 4096], BF16)
            yf = yf_p.tile([80, 8192], BF16)
            slab = slab_p.tile([128, 18 * 144], F8)
            slabs.append(slab)

            def do_a(k):
                psA = psAT_p.tile([128, 512], F32, tag="psAT", name=f"psA{k}")
                for q in range(4):
                    pp = k * 4 + q
                    nc.tensor.matmul(psA[:, q * 128:(q + 1) * 128],
                                     xpat[:, pp * 128:(pp + 1) * 128], winT,
                                     start=True, stop=True)
                nc.scalar.copy(abuf[:, k * 512:(k + 1) * 512], psA)

            def do_b(k):
                h2, xh, d0 = k // 4, (k // 2) % 2, 4 * (k % 2)
                psB = psB_p.tile([80, 1024], F32, tag="psB", name=f"psB{k}")
                for pl in range(2):
                    nc.tensor.matmul(
                        psB[:, pl * 512:(pl + 1) * 512],
                        f80l[pl * 64:(pl + 1) * 64, :],
                        abuf[pl * 64:(pl + 1) * 64, k * 512:(k + 1) * 512],
                        start=True, stop=True)
                qq0 = ((h2 * 2 + xh) * 8 + d0) * 2
                dst = bass.AP(tensor=yf.tensor, offset=yf.offset + qq0 * 128,
                              ap=[yf.ap[0], [128, 2], [256, 4], [1, 128]])
                src = bass.AP(tensor=psB.tensor, offset=psB.offset,
                              ap=[psB.ap[0], [512, 2], [128, 4], [1, 128]])
                ssrc = bass.AP(tensor=s80t.tensor, offset=s80t.offset,
                               ap=[s80t.ap[0], [0, 2], [128, 4], [1, 128]])
                nc.vector.tensor_tensor(dst, src, ssrc, MULT)

            def do_t(j):
                h2, xh, dh = j // 4, (j // 2) % 2, j % 2
                if True:
                    if True:
                        psT = psAT_p.tile([128, 512], F32, tag="psAT")
                        for dd in range(4):
                            for pl in range(2):
                                d = dh * 4 + dd
                                qq = ((h2 * 2 + xh) * 8 + d) * 2 + pl
                                nc.tensor.matmul(
                                    psT[:, (dd * 2 + pl) * 64:(dd * 2 + pl) * 64 + 64],
                                    yf[:, qq * 128:(qq + 1) * 128], g80m,
                                    start=True, stop=True)
                        gel = gelu_p.tile([64, 512], BF16)
                        nc.scalar.activation(gel, psT[0:64, :],
                                             mybir.ActivationFunctionType.Gelu)
                        # gate: (gelu x1) * x2 -> slab rows 1+8*h2, half xh
                        sps = slab.ap[0][0]
                        dst = bass.AP(
                            tensor=slab.tensor,
                            offset=slab.offset + xh * 64 * sps
                            + (1 + 8 * h2) * 144 + 1 + dh * 64,
                            ap=[[sps, 64], [16, 4], [8, 2], [144, 8], [1, 8]])
                        src0 = bass.AP(
                            tensor=gel.tensor, offset=gel.offset,
                            ap=[[gel.ap[0][0], 64], [128, 4], [64, 2], [8, 8], [1, 8]])
                        tps = psT.ap[0][0]
                        src1 = bass.AP(
                            tensor=psT.tensor, offset=psT.offset + 64 * tps,
                            ap=[[tps, 64], [128, 4], [64, 2], [8, 8], [1, 8]])
                        nc.vector.tensor_tensor(dst, src0, src1, MULT)

            for k in range(2):
                do_a(k)
            for k in range(2, 8):
                do_a(k)
                do_b(k - 2)
                if k >= 5:
                    do_t(k - 5)
            do_b(6)
            do_t(3)
            do_b(7)
            for j in range(4, 8):
                do_t(j)

            # halo wiring
            sl3 = slab.rearrange("p (r c) -> p r c", c=144)
            nc.gpsimd.memset(sl3[0:64, 1:17, 0:1], 0.0)
            nc.gpsimd.memset(sl3[64:128, 1:17, 129:130], 0.0)
            nc.sync.dma_start(out=sl3[0:64, 1:17, 129:130],
                              in_=sl3[64:128, 1:17, 1:2])
            nc.sync.dma_start(out=sl3[64:128, 1:17, 0:1],
                              in_=sl3[0:64, 1:17, 128:129])
            if t == 0:
                nc.vector.memset(sl3[:, 0:1, :], 0.0)
            else:
                prev3 = slabs[t - 1].rearrange("p (r c) -> p r c", c=144)
                nc.vector.tensor_copy(prev3[:, 17:18, :], sl3[:, 1:2, :])
                nc.vector.tensor_copy(sl3[:, 0:1, :], prev3[:, 16:17, :])
            if t == n_bands - 1:
                nc.vector.memset(sl3[:, 17:18, :], 0.0)

        gws = {}

        def do_psw(t, w):
            slab = slabs[t]
            gw = gws[t]
            psW = psWO_p.tile([128, 512], F32, tag="psWO")

            def off(dy, dx):
                return (1 + w * 4 + dy) * 144 + (1 + dx)

            for j in range(3):
                dx = j - 1
                lhs = bass.AP(tensor=wk9.tensor,
                              offset=wk9.offset + j * 256,
                              ap=[wk9.ap[0], [128, 2], [1, 128]])
                rhs = bass.AP(
                    tensor=slab.tensor,
                    offset=slab.offset + off(-1, dx),
                    ap=[slab.ap[0], [144, 2], [144, 4], [1, 128]])
                nc.tensor.matmul(psW, lhs, rhs,
                                 start=(j == 0), stop=False,
                                 perf_mode=mybir.MatmulPerfMode.DoubleRow,
                                 skip_group_check=True)
            for j in range(3):
                dx = j - 1
                rhs8 = bass.AP(
                    tensor=slab.tensor,
                    offset=slab.offset + off(1, dx),
                    ap=[slab.ap[0], [144, 4], [1, 128]])
                nc.tensor.matmul(psW, wk9[:, 768 + j * 128:768 + (j + 1) * 128],
                                 rhs8, start=False, stop=(j == 2),
                                 skip_group_check=True)
            if w % 2 == 0:
                nc.scalar.copy(gw[:, w * 512:(w + 1) * 512], psW)
            else:
                nc.vector.tensor_copy(gw[:, w * 512:(w + 1) * 512], psW)

        def tail_pre(t):
            # psW tiles 0..2 only touch slab rows 0..14: ready without the
            # next band's gates (row 17).
            gws[t] = gw_p.tile([128, 2048], BF16, name=f'gw{t}', tag='gw')
            for w in range(3):
                do_psw(t, w)

        def tail_post(t):
            xbf = xbfs[t]
            gw = gws[t]
            y0 = t * BAND
            do_psw(t, 3)

            # w_out + residual (identity matmul on bf16 input), evict fp32
            outb = outb_p.tile([128, BAND * W], F32)
            for xh in range(2):
                for w in range(4):
                    psO = psWO_p.tile([128, 512], F32, tag="psWO")
                    nc.tensor.matmul(
                        psO, woT2[xh * 64:(xh + 1) * 64, :],
                        gw[xh * 64:(xh + 1) * 64, w * 512:(w + 1) * 512],
                        start=True, stop=False, skip_group_check=True)
                    rx = bass.AP(tensor=xbf.tensor,
                                 offset=xbf.offset + w * 4 * 256 + xh * 128,
                                 ap=[xbf.ap[0], [256, 4], [1, 128]])
                    nc.tensor.matmul(psO, ident, rx,
                                     start=False, stop=True,
                                     skip_group_check=True)
                    dst = bass.AP(tensor=outb.tensor,
                                  offset=outb.offset + w * 4 * 256 + xh * 128,
                                  ap=[outb.ap[0], [256, 4], [1, 128]])
                    src = bass.AP(tensor=psO.tensor, offset=psO.offset,
                                  ap=[psO.ap[0], [128, 4], [1, 128]])
                    nc.scalar.copy(dst, src)
            nc.sync.dma_start(out=out_d[:, y0:y0 + BAND, :], in_=outb)

        for t in range(n_bands):
            front(t)
            if t > 0:
                tail_pre(t - 1)
                tail_post(t - 1)
        tail_pre(n_bands - 1)
        tail_post(n_bands - 1)

    if legalize:
        _spill_matmul_waits(nc)
    return nc


def _spill_matmul_waits(nc):
    """Walrus encodes at most ONE sync-wait per compute-engine ISA
    instruction; split extras into standalone EventSemaphore carriers."""
    import concourse.mybir as mb
    skip = (mb.InstEventSemaphore,)
    n = [0]
    for f in nc.m.functions:
        for bb in f.blocks:
            out = []
            for inst in bb.instructions:
                si = inst.sync_info
                if (si is not None and len(si.on_wait) > 1
                        and not isinstance(inst, skip)
                        and getattr(inst, 'engine', None) is not None):
                    extra, keep = si.on_wait[:-1], si.on_wait[-1:]
                    for wv in extra:
                        n[0] += 1
                        carrier = mb.InstEventSemaphore(
                            name=f"I-waitfix-{n[0]}", ins=[], outs=[])
                        carrier.engine = inst.engine
                        carrier.sync_info = mb.SyncInfo(
                            on_wait=[wv], on_update=[])
                        out.append(carrier)
                    si.on_wait = keep
                out.append(inst)
            bb.instructions = out


# --------------------------------------------------------------------------
# public entry point
# --------------------------------------------------------------------------

_CACHE = {}


def _get_nc():
    if "nc" not in _CACHE:
        nc = bass.Bass("TRN2", target_bir_lowering=False, debug=False)
        build_kernel(nc, n_rows=H)
        _CACHE["nc"] = nc
    return _CACHE["nc"]


def kernel(x, fft_filt, w_in, w_before, w_dw, w_out):
    x = np.asarray(x, dtype=np.float32)
    winT, f80l, s80t, g80m, wk9, woT2 = _prep_weights(
        np.asarray(fft_filt, np.float32), np.asarray(w_in, np.float32),
        np.asarray(w_before, np.float32), np.asarray(w_dw, np.float32),
        np.asarray(w_out, np.float32))

    nc = _get_nc()
    in_maps = []
    for i in range(N_CORES):
        in_maps.append({
            "x": np.ascontiguousarray(x[i]),
            "winT": winT, "f80l": f80l, "s80t": s80t,
            "g80m": g80m, "wk9": wk9, "woT2": woT2,
        })
    res = run_bass_kernel_spmd(nc, in_maps, list(range(N_CORES)))
    out = np.stack([res.results[i]["out"] for i in range(N_CORES)], axis=0)
    return out.astype(np.float32)


# revision 4
# speedup vs baseline: 1.0105x; 1.0105x over previous
"""DFFN Trainium2 kernel v3: restructured for TimelineSim cost model.

Data-parallel over batch: 8 images, one per NeuronCore.

Pipeline per 16-row band (vs the v1 per-channel-matmul design):
  - input loads via gpsimd cast-DMA straight to bf16 (no fp32 copy in SBUF)
  - proj_in runs flipped (patch-pair chunks stationary) -> pixel comps on
    PSUM partitions
  - the patch FFT*filt*iFFT factorizes as G80 . diag(s_c) . F80 with
    CHANNEL-SHARED 80-dim real-spectrum transforms; only diag(s_c) is
    per-channel and it is a single elementwise multiply (B2). This replaces
    2048 per-channel matmuls with 48 shared-weight matmuls per band.
  - T' merges the inverse transform with the return-to-channel-partitions
    transpose: lhsT = scaled spectrum chunk, rhs = G80^T moving.
  - gelu reads PSUM x1 rows, gate multiplies by x2 rows and scatters
    DIRECTLY into the halo slab (no separate transpose eviction).
  - depthwise 3x3 folds into the before_dwconv matmul: psW accumulates 9
    matmuls of diag(wdw_k)@w_before against shifted slab windows.
  - residual rides the w_out PSUM group as an extra identity matmul on the
    bf16 input; a single Act eviction writes the final fp32 band.
"""

import sys

sys.path.insert(0, "/opt/trn_rl_repo")

import numpy as np
import ml_dtypes
from contextlib import ExitStack

import concourse.bass as bass
import concourse.mybir as mybir
import concourse.tile as tile
from concourse.bass_utils import run_bass_kernel_spmd
from concourse.masks import make_identity

F32 = mybir.dt.float32
BF16 = mybir.dt.bfloat16
F8 = mybir.dt.float8e4
BF = ml_dtypes.bfloat16
F8NP = ml_dtypes.float8_e4m3fn

B, C, H, W = 8, 128, 256, 256
HALF = C // 2
P = 8
BAND = 16
N_CORES = 8


# --------------------------------------------------------------------------
# host-side weight preprocessing
# --------------------------------------------------------------------------

def _f80_g80():
    """Shared real-spectrum transforms: F80 [80,64] pixel->spectrum slots,
    G80 [64,80] spectrum slots->pixel. Slot s=(u*5+v)*2+ri."""
    E = np.eye(P * P).reshape(P * P, P, P)
    FB = np.fft.rfft2(E)                       # [64, 8, 5]
    F80 = np.zeros((80, 64))
    G80 = np.zeros((64, 80))
    for u in range(8):
        for v in range(5):
            for ri in range(2):
                s = (u * 5 + v) * 2 + ri
                F80[s, :] = FB[:, u, v].real if ri == 0 else FB[:, u, v].imag
                d = np.zeros((8, 5), dtype=complex)
                d[u, v] = 1.0 if ri == 0 else 1.0j
                G80[:, s] = np.fft.irfft2(d, s=(P, P)).reshape(64)
    return F80, G80


def _prep_weights(fft_filt, w_in, w_before, w_dw, w_out):
    F80, G80 = _f80_g80()

    winT = np.ascontiguousarray(w_in.T)                   # [cin, cout]

    f80l = np.vstack([F80.T, F80.T])                      # [128, 80]

    s80 = np.zeros((80, C))                               # [slot, c]
    for u in range(8):
        for v in range(5):
            for ri in range(2):
                s80[(u * 5 + v) * 2 + ri, :] = fft_filt[:, u, v]
    s80t = np.tile(s80, (1, 4))                           # [80, 512]

    g80m = np.ascontiguousarray(G80.T)                    # [80, 64]

    # 9 taps: Wk = diag(wdw[:,k]) @ w_before ; lhsT = blockdiag(Wk^T, Wk^T).
    # fp8 DoubleRow pairs taps (0,1),(2,3),(4,5),(6,7) two-major + single 8.
    # Weights scaled x64 into fp8 normal range; woT2 carries the /64.
    wdw9 = w_dw.reshape(HALF, 9)
    wk1 = np.zeros((9, 128, 128))
    for k in range(9):
        WkT = (wdw9[:, k:k + 1] * w_before).T             # [cin=64, cout=64]
        wk1[k, 0:64, 0:64] = WkT
        wk1[k, 64:128, 64:128] = WkT
    wkp = np.zeros((128, 9 * 128))
    for j in range(3):
        wkp[:, j * 256:j * 256 + 128] = wk1[j] * 64.0
        wkp[:, j * 256 + 128:j * 256 + 256] = wk1[j + 3] * 64.0
    for j in range(3):
        wkp[:, 768 + j * 128:768 + (j + 1) * 128] = wk1[6 + j] * 64.0

    woT2 = np.vstack([w_out.T, w_out.T]) / 64.0           # [128, 128]

    return (winT.astype(BF), f80l.astype(BF), s80t.astype(BF),
            g80m.astype(BF), wkp.astype(F8NP), woT2.astype(BF))


# --------------------------------------------------------------------------
# the tile kernel (per core, one image)
# --------------------------------------------------------------------------

def build_kernel(nc, n_rows=H, legalize=True):
    x_d = nc.dram_tensor("x", [C, n_rows, W], F32, kind="ExternalInput").ap()
    winT_d = nc.dram_tensor("winT", [128, 128], BF16, kind="ExternalInput").ap()
    f80l_d = nc.dram_tensor("f80l", [128, 80], BF16, kind="ExternalInput").ap()
    s80t_d = nc.dram_tensor("s80t", [80, 512], BF16, kind="ExternalInput").ap()
    g80m_d = nc.dram_tensor("g80m", [80, 64], BF16, kind="ExternalInput").ap()
    wk9_d = nc.dram_tensor("wk9", [128, 9 * 128], F8, kind="ExternalInput").ap()
    woT2_d = nc.dram_tensor("woT2", [128, 128], BF16, kind="ExternalInput").ap()
    out_d = nc.dram_tensor("out", [C, n_rows, W], F32, kind="ExternalOutput").ap()

    n_bands = n_rows // BAND
    MULT = mybir.AluOpType.mult

    with tile.TileContext(nc) as tc, ExitStack() as ctx:
        singles = ctx.enter_context(tc.tile_pool(name="singles", bufs=1))
        xbf_p = ctx.enter_context(tc.tile_pool(name="xbf", bufs=4))
        xpat_p = ctx.enter_context(tc.tile_pool(name="xpat", bufs=3))
        abuf_p = ctx.enter_context(tc.tile_pool(name="abuf", bufs=3))
        yf_p = ctx.enter_context(tc.tile_pool(name="yf", bufs=2))
        gelu_p = ctx.enter_context(tc.tile_pool(name="gelu", bufs=4))
        gw_p = ctx.enter_context(tc.tile_pool(name="gw", bufs=3))
        slab_p = ctx.enter_context(tc.tile_pool(name="slab", bufs=4))
        outb_p = ctx.enter_context(tc.tile_pool(name="outb", bufs=2))

        psAT_p = ctx.enter_context(tc.tile_pool(name="psAT", bufs=2, space="PSUM"))
        psB_p = ctx.enter_context(tc.tile_pool(name="psB", bufs=2, space="PSUM"))
        psWO_p = ctx.enter_context(tc.tile_pool(name="psWO", bufs=2, space="PSUM"))

        # ---- weights ----
        winT = singles.tile([128, 128], BF16)
        nc.sync.dma_start(out=winT, in_=winT_d)
        f80l = singles.tile([128, 80], BF16)
        nc.sync.dma_start(out=f80l, in_=f80l_d)
        s80t = singles.tile([80, 512], BF16)
        nc.sync.dma_start(out=s80t, in_=s80t_d)
        g80m = singles.tile([80, 64], BF16)
        nc.sync.dma_start(out=g80m, in_=g80m_d)
        wk9 = singles.tile([128, 9 * 128], F8)
        nc.sync.dma_start(out=wk9, in_=wk9_d)
        woT2 = singles.tile([128, 128], BF16)
        nc.sync.dma_start(out=woT2, in_=woT2_d)
        ident = singles.tile([128, 128], BF16)
        make_identity(nc, ident)

        xbfs = []
        slabs = []

        def front(t, midwork=None):
            y0 = t * BAND
            # 0. cast-DMA: DRAM fp32 -> SBUF bf16, row-major
            xbf = xbf_p.tile([128, BAND * W], BF16)
            nc.gpsimd.dma_start(out=xbf, in_=x_d[:, y0:y0 + BAND, :])
            xbfs.append(xbf)

            # reorder to patch-pair-major: xpat col = pp*128 + pl*64 + i*8 + j
            xpat = xpat_p.tile([128, BAND * W], BF16)
            for h2 in range(2):
                src = bass.AP(tensor=xbf.tensor, offset=xbf.offset + h2 * 2048,
                              ap=[xbf.ap[0], [16, 16], [8, 2], [256, 8], [1, 8]])
                dst = bass.AP(tensor=xpat.tensor, offset=xpat.offset + h2 * 2048,
                              ap=[xpat.ap[0], [128, 16], [64, 2], [8, 8], [1, 8]])
                nc.gpsimd.tensor_copy(dst, src)

            # A / B1+B2 / T'+gelu+gate, software-pipelined on PE
            abuf = abuf_p.tile([128, 4096], BF16)
            yf = yf_p.tile([80, 8192], BF16)
            slab = slab_p.tile([128, 18 * 144], F8)
            slabs.append(slab)

            def do_a(k):
                psA = psAT_p.tile([128, 512], F32, tag="psAT", name=f"psA{k}")
                for q in range(4):
                    pp = k * 4 + q
                    nc.tensor.matmul(psA[:, q * 128:(q + 1) * 128],
                                     xpat[:, pp * 128:(pp + 1) * 128], winT,
                                     start=True, stop=True)
                nc.scalar.copy(abuf[:, k * 512:(k + 1) * 512], psA)

            def do_b(k):
                h2, xh, d0 = k // 4, (k // 2) % 2, 4 * (k % 2)
                psB = psB_p.tile([80, 1024], F32, tag="psB", name=f"psB{k}")
                for pl in range(2):
                    nc.tensor.matmul(
                        psB[:, pl * 512:(pl + 1) * 512],
                        f80l[pl * 64:(pl + 1) * 64, :],
                        abuf[pl * 64:(pl + 1) * 64, k * 512:(k + 1) * 512],
                        start=True, stop=True)
                qq0 = ((h2 * 2 + xh) * 8 + d0) * 2
                dst = bass.AP(tensor=yf.tensor, offset=yf.offset + qq0 * 128,
                              ap=[yf.ap[0], [128, 2], [256, 4], [1, 128]])
                src = bass.AP(tensor=psB.tensor, offset=psB.offset,
                              ap=[psB.ap[0], [512, 2], [128, 4], [1, 128]])
                ssrc = bass.AP(tensor=s80t.tensor, offset=s80t.offset,
                               ap=[s80t.ap[0], [0, 2], [128, 4], [1, 128]])
                nc.vector.tensor_tensor(dst, src, ssrc, MULT)

            def do_t(j):
                h2, xh, dh = j // 4, (j // 2) % 2, j % 2
                if True:
                    if True:
                        psT = psAT_p.tile([128, 512], F32, tag="psAT")
                        for dd in range(4):
                            for pl in range(2):
                                d = dh * 4 + dd
                                qq = ((h2 * 2 + xh) * 8 + d) * 2 + pl
                                nc.tensor.matmul(
                                    psT[:, (dd * 2 + pl) * 64:(dd * 2 + pl) * 64 + 64],
                                    yf[:, qq * 128:(qq + 1) * 128], g80m,
                                    start=True, stop=True)
                        gel = gelu_p.tile([64, 512], BF16)
                        nc.scalar.activation(gel, psT[0:64, :],
                                             mybir.ActivationFunctionType.Gelu)
                        # gate: (gelu x1) * x2 -> slab rows 1+8*h2, half xh
                        sps = slab.ap[0][0]
                        dst = bass.AP(
                            tensor=slab.tensor,
                            offset=slab.offset + xh * 64 * sps
                            + (1 + 8 * h2) * 144 + 1 + dh * 64,
                            ap=[[sps, 64], [16, 4], [8, 2], [144, 8], [1, 8]])
                        src0 = bass.AP(
                            tensor=gel.tensor, offset=gel.offset,
                            ap=[[gel.ap[0][0], 64], [128, 4], [64, 2], [8, 8], [1, 8]])
                        tps = psT.ap[0][0]
                        src1 = bass.AP(
                            tensor=psT.tensor, offset=psT.offset + 64 * tps,
                            ap=[[tps, 64], [128, 4], [64, 2], [8, 8], [1, 8]])
                        nc.vector.tensor_tensor(dst, src0, src1, MULT)

            for k in range(2):
                do_a(k)
            for k in range(2, 8):
                do_a(k)
                do_b(k - 2)
                if k >= 5:
                    do_t(k - 5)
            do_b(6)
            do_t(3)
            do_b(7)
            for j in range(4, 8):
                do_t(j)

            # halo wiring
            sl3 = slab.rearrange("p (r c) -> p r c", c=144)
            nc.gpsimd.memset(sl3[0:64, 1:17, 0:1], 0.0)
            nc.gpsimd.memset(sl3[64:128, 1:17, 129:130], 0.0)
            nc.sync.dma_start(out=sl3[0:64, 1:17, 129:130],
                              in_=sl3[64:128, 1:17, 1:2])
            nc.sync.dma_start(out=sl3[64:128, 1:17, 0:1],
                              in_=sl3[0:64, 1:17, 128:129])
            if t == 0:
                nc.vector.memset(sl3[:, 0:1, :], 0.0)
            else:
                prev3 = slabs[t - 1].rearrange("p (r c) -> p r c", c=144)
                nc.vector.tensor_copy(prev3[:, 17:18, :], sl3[:, 1:2, :])
                nc.vector.tensor_copy(sl3[:, 0:1, :], prev3[:, 16:17, :])
            if t == n_bands - 1:
                nc.vector.memset(sl3[:, 17:18, :], 0.0)

        gws = {}

        def do_psw(t, w):
            slab = slabs[t]
            gw = gws[t]
            psW = psWO_p.tile([128, 512], F32, tag="psWO")

            def off(dy, dx):
                return (1 + w * 4 + dy) * 144 + (1 + dx)

            for j in range(3):
                dx = j - 1
                lhs = bass.AP(tensor=wk9.tensor,
                              offset=wk9.offset + j * 256,
                              ap=[wk9.ap[0], [128, 2], [1, 128]])
                rhs = bass.AP(
                    tensor=slab.tensor,
                    offset=slab.offset + off(-1, dx),
                    ap=[slab.ap[0], [144, 2], [144, 4], [1, 128]])
                nc.tensor.matmul(psW, lhs, rhs,
                                 start=(j == 0), stop=False,
                                 perf_mode=mybir.MatmulPerfMode.DoubleRow,
                                 skip_group_check=True)
            for j in range(3):
                dx = j - 1
                rhs8 = bass.AP(
                    tensor=slab.tensor,
                    offset=slab.offset + off(1, dx),
                    ap=[slab.ap[0], [144, 4], [1, 128]])
                nc.tensor.matmul(psW, wk9[:, 768 + j * 128:768 + (j + 1) * 128],
                                 rhs8, start=False, stop=(j == 2),
                                 skip_group_check=True)
            if w % 2 == 0:
                nc.scalar.copy(gw[:, w * 512:(w + 1) * 512], psW)
            else:
                nc.vector.tensor_copy(gw[:, w * 512:(w + 1) * 512], psW)

        def tail_pre(t):
            # psW tiles 0..2 only touch slab rows 0..14: ready without the
            # next band's gates (row 17).
            gws[t] = gw_p.tile([128, 2048], BF16, name=f'gw{t}', tag='gw')
            for w in range(3):
                do_psw(t, w)

        def tail_post(t):
            xbf = xbfs[t]
            gw = gws[t]
            y0 = t * BAND
            do_psw(t, 3)

            # w_out + residual (identity matmul on bf16 input), evict fp32
            outb = outb_p.tile([128, BAND * W], F32)
            for xh, w in [(0, 0), (0, 1), (0, 2), (0, 3),
                          (1, 0), (1, 1), (1, 2), (1, 3)]:
                if True:
                    psO = psWO_p.tile([128, 512], F32, tag="psWO")
                    nc.tensor.matmul(
                        psO, woT2[xh * 64:(xh + 1) * 64, :],
                        gw[xh * 64:(xh + 1) * 64, w * 512:(w + 1) * 512],
                        start=True, stop=False, skip_group_check=True)
                    rx = bass.AP(tensor=xbf.tensor,
                                 offset=xbf.offset + w * 4 * 256 + xh * 128,
                                 ap=[xbf.ap[0], [256, 4], [1, 128]])
                    nc.tensor.matmul(psO, ident, rx,
                                     start=False, stop=True,
                                     skip_group_check=True)
                    dst = bass.AP(tensor=outb.tensor,
                                  offset=outb.offset + w * 4 * 256 + xh * 128,
                                  ap=[outb.ap[0], [256, 4], [1, 128]])
                    src = bass.AP(tensor=psO.tensor, offset=psO.offset,
                                  ap=[psO.ap[0], [128, 4], [1, 128]])
                    nc.scalar.copy(dst, src)
            nc.sync.dma_start(out=out_d[:, y0:y0 + BAND, :], in_=outb)

        for t in range(n_bands):
            front(t)
            if t > 0:
                tail_pre(t - 1)
                tail_post(t - 1)
        tail_pre(n_bands - 1)
        tail_post(n_bands - 1)

    if legalize:
        _spill_matmul_waits(nc)
    return nc


def _spill_matmul_waits(nc):
    """Walrus encodes at most ONE sync-wait per compute-engine ISA
    instruction; split extras into standalone EventSemaphore carriers."""
    import concourse.mybir as mb
    skip = (mb.InstEventSemaphore,)
    n = [0]
    for f in nc.m.functions:
        for bb in f.blocks:
            out = []
            for inst in bb.instructions:
                si = inst.sync_info
                if (si is not None and len(si.on_wait) > 1
                        and not isinstance(inst, skip)
                        and getattr(inst, 'engine', None) is not None):
                    extra, keep = si.on_wait[:-1], si.on_wait[-1:]
                    for wv in extra:
                        n[0] += 1
                        carrier = mb.InstEventSemaphore(
                            name=f"I-waitfix-{n[0]}", ins=[], outs=[])
                        carrier.engine = inst.engine
                        carrier.sync_info = mb.SyncInfo(
                            on_wait=[wv], on_update=[])
                        out.append(carrier)
                    si.on_wait = keep
                out.append(inst)
            bb.instructions = out


# --------------------------------------------------------------------------
# public entry point
# --------------------------------------------------------------------------

_CACHE = {}


def _get_nc():
    if "nc" not in _CACHE:
        nc = bass.Bass("TRN2", target_bir_lowering=False, debug=False)
        build_kernel(nc, n_rows=H)
        _CACHE["nc"] = nc
    return _CACHE["nc"]


def kernel(x, fft_filt, w_in, w_before, w_dw, w_out):
    x = np.asarray(x, dtype=np.float32)
    winT, f80l, s80t, g80m, wk9, woT2 = _prep_weights(
        np.asarray(fft_filt, np.float32), np.asarray(w_in, np.float32),
        np.asarray(w_before, np.float32), np.asarray(w_dw, np.float32),
        np.asarray(w_out, np.float32))

    nc = _get_nc()
    in_maps = []
    for i in range(N_CORES):
        in_maps.append({
            "x": np.ascontiguousarray(x[i]),
            "winT": winT, "f80l": f80l, "s80t": s80t,
            "g80m": g80m, "wk9": wk9, "woT2": woT2,
        })
    res = run_bass_kernel_spmd(nc, in_maps, list(range(N_CORES)))
    out = np.stack([res.results[i]["out"] for i in range(N_CORES)], axis=0)
    return out.astype(np.float32)
